# revision 7
# baseline (speedup 1.0000x reference)
"""AFNO1D Trainium2 kernel (8 NeuronCores, SPMD over the token axis).

Math: the reference's DHT/flip/block-matmul soup folds exactly into:
  o1 = relu(x @ MA + flip_B(x) @ MC + b1)        (MA/MC dense 1024x1024, H1024 folded in)
  o2 = o1 . W2A + flip_B(o1) . W2C + b2          (per-block 128x128)
  z  = softshrink(o2, 0.01) @ (H128 / 2^24)      (per-block)
  out = z + x
flip_B is the batch permutation k -> (4-k)%4, handled with a batch-parity basis
(E0=x0, E13=(x1+x3)/2, E2=x2, O13=(x1-x3)/2) so each layer needs one matmul
stream per unit. |z| ~ 1e-8 * |x|, so bf16 compute is far inside the tolerance.
Each core takes 512 of the 4096 tokens; there is no cross-token coupling, so no
collectives are needed.
"""

import numpy as np
import ml_dtypes
from contextlib import ExitStack

import concourse.bass as bass
import concourse.tile as tile
import concourse.mybir as mybir
from concourse import bacc
from concourse.bass_utils import run_bass_kernel_spmd

NB, BS, HID = 8, 128, 1024
B, N = 4, 4096
NCORES = 8
TOK = N // NCORES            # tokens per core
ROWS = B * TOK               # 2048 rows per core (4 units x 512 tokens)
NUMEL = B * N * HID          # 2^24 (idht normalizes by total numel)
LAM = 0.01
RC = 512                     # row-chunk (matmul moving free dim)
NRC = ROWS // RC             # 4

F32 = mybir.dt.float32
BF16 = mybir.dt.bfloat16


def _cas(n):
    idx = np.arange(n)
    ang = 2.0 * np.pi * np.outer(idx, idx) / n
    return np.cos(ang) + np.sin(ang)


def _flp(a):
    return np.roll(a[::-1], 1, axis=0)


def _fold_weights(w, H128):
    """w [2, nb, i, o] -> WA, WC [nb, i, o] float64 so that
    CM(x, w[0]) + CM(x, w[1]) = x . WA + flip_B(x) . WC  (per block)."""
    WA = np.zeros((NB, BS, BS))
    WC = np.zeros((NB, BS, BS))
    for j in range(2):
        y = w[j].astype(np.float64)
        Y = y @ H128
        yf = _flp(y)
        WA += 0.5 / NUMEL * np.einsum('ji,bio,ok->bjk', H128, Y + yf, H128)
        WC += 0.5 / NUMEL * (Y - yf) @ H128
    return WA, WC


def _prep_weights(w1, b1, w2, b2):
    H1024 = _cas(HID)
    H128 = _cas(BS)
    W1A, W1C = _fold_weights(w1, H128)
    W2A, W2C = _fold_weights(w2, H128)

    # Fold H1024 into layer 1 -> dense [1024, 1024]
    MA = np.zeros((HID, HID))
    MC = np.zeros((HID, HID))
    for b in range(NB):
        cols = slice(b * BS, (b + 1) * BS)
        MA[:, cols] = H1024[:, cols] @ W1A[b]
        MC[:, cols] = H1024[:, cols] @ W1C[b]
    Mp, Mm = MA + MC, MA - MC

    W2sum = W2A + W2C            # units 0,2 (flip-invariant)
    W2ph = 0.5 * (W2A + W2C)     # e-path (s = o1[1]+o1[3])
    W2mh = 0.5 * (W2A - W2C)     # o-path (d = o1[1]-o1[3])

    def sb_w(M):  # [1024, out] -> SBUF layout [128, in_hi, out]
        return np.ascontiguousarray(
            M.reshape(NB, BS, -1).transpose(1, 0, 2).astype(ml_dtypes.bfloat16))

    def sb_blk(W):  # [nb, i, o] -> [128, nb, o]
        return np.ascontiguousarray(
            W.transpose(1, 0, 2).astype(ml_dtypes.bfloat16))

    return {
        "Mp": sb_w(Mp), "Mm": sb_w(Mm),
        "W2sum": sb_blk(W2sum), "W2ph": sb_blk(W2ph), "W2mh": sb_blk(W2mh),
        "H128s": np.ascontiguousarray((H128 / NUMEL).astype(ml_dtypes.bfloat16)),
        "b1": np.ascontiguousarray(b1[0].astype(np.float32).T),   # [128, 8]
        "b2": np.ascontiguousarray(b2[0].astype(np.float32).T),   # [128, 8]
    }


def build_nc():
    nc = bacc.Bacc("TRN2", target_bir_lowering=False, debug=False)

    xp_ext = nc.declare_dram_parameter("xp", [BS, NB, ROWS], BF16, isOutput=False)
    mp_ext = nc.declare_dram_parameter("Mp", [BS, NB, HID], BF16, isOutput=False)
    mm_ext = nc.declare_dram_parameter("Mm", [BS, NB, HID], BF16, isOutput=False)
    w2s_ext = nc.declare_dram_parameter("W2sum", [BS, NB, BS], BF16, isOutput=False)
    w2p_ext = nc.declare_dram_parameter("W2ph", [BS, NB, BS], BF16, isOutput=False)
    w2m_ext = nc.declare_dram_parameter("W2mh", [BS, NB, BS], BF16, isOutput=False)
    h_ext = nc.declare_dram_parameter("H128s", [BS, BS], BF16, isOutput=False)
    b1_ext = nc.declare_dram_parameter("b1", [BS, NB], F32, isOutput=False)
    b2_ext = nc.declare_dram_parameter("b2", [BS, NB], F32, isOutput=False)
    xr_ext = nc.declare_dram_parameter("xr", [BS, NB, 2 * TOK], BF16, isOutput=False)
    out_ext = nc.declare_dram_parameter("out", [BS, NB, ROWS], BF16, isOutput=True)

    RELU = mybir.ActivationFunctionType.Relu
    IDENT = mybir.ActivationFunctionType.Identity
    ADD = mybir.AluOpType.add
    SUB = mybir.AluOpType.subtract
    MAX = mybir.AluOpType.max
    MIN = mybir.AluOpType.min

    with tile.TileContext(nc) as tc:
        with ExitStack() as ctx:
            wpool = ctx.enter_context(tc.tile_pool(name="w", bufs=1))
            apool = ctx.enter_context(tc.tile_pool(name="act", bufs=1))
            tpool = ctx.enter_context(tc.tile_pool(name="tmp", bufs=2))
            opool = ctx.enter_context(tc.tile_pool(name="outb", bufs=3))
            ppool = ctx.enter_context(tc.tile_pool(name="ps", bufs=8, space="PSUM"))

            # ---- resident tensors ----
            xp = apool.tile([BS, NB, ROWS], BF16)      # input, parity units
            nc.sync.dma_start(xp[:], xp_ext[:])
            Mp = wpool.tile([BS, NB, HID], BF16)
            nc.sync.dma_start(Mp[:], mp_ext[:])
            Mm = wpool.tile([BS, NB, HID], BF16)
            nc.sync.dma_start(Mm[:], mm_ext[:])
            W2s = wpool.tile([BS, NB, BS], BF16)
            nc.sync.dma_start(W2s[:], w2s_ext[:])
            W2p = wpool.tile([BS, NB, BS], BF16)
            nc.sync.dma_start(W2p[:], w2p_ext[:])
            W2m = wpool.tile([BS, NB, BS], BF16)
            nc.sync.dma_start(W2m[:], w2m_ext[:])
            H128s = wpool.tile([BS, BS], BF16)
            nc.sync.dma_start(H128s[:], h_ext[:])
            b1 = wpool.tile([BS, NB], F32)
            nc.sync.dma_start(b1[:], b1_ext[:])
            b2 = wpool.tile([BS, NB], F32)
            nc.sync.dma_start(b2[:], b2_ext[:])

            o1 = apool.tile([BS, NB, ROWS], BF16)      # relu output (unit space)
            sd = apool.tile([BS, NB, 2 * TOK], BF16)   # s=o1[1]+o1[3] | d=o1[1]-o1[3]
            xr = apool.tile([BS, NB, 2 * TOK], BF16)   # residual x1 | x3
            nc.sync.dma_start(xr[:], xr_ext[:])

            def rows_u(u):  # row slice of unit/batch u
                return bass.ds(u * TOK, TOK)

            # ---- stage L01: o1 = relu(units @ {Mp,Mm} + b1) ----
            for m in range(NB):
                ps = [ppool.tile([BS, RC], F32, tag="ps", name=f"ps_l01_{m}_{u}") for u in range(4)]
                for k in range(NB):
                    for u in range(4):
                        w = Mm if u == 3 else Mp
                        nc.tensor.matmul(
                            ps[u][:], w[:, k, bass.ts(m, BS)], xp[:, k, rows_u(u)],
                            start=(k == 0), stop=(k == NB - 1))
                # units 0,2: relu(psum + b1)
                for u in (0, 2):
                    nc.scalar.activation(
                        o1[:, m, rows_u(u)], ps[u][:], RELU, bias=b1[:, m:m + 1])
                # units 1,3: e +/- o then relu+bias (DVE reads one PSUM max,
                # so bounce o through SBUF on ACT first)
                osb = tpool.tile([BS, RC], F32, tag="osb", name=f"osb_{m}")
                nc.scalar.activation(osb[:], ps[3][:], IDENT)
                t13 = [tpool.tile([BS, RC], F32, tag="t13", name=f"t13_{m}_{i}") for i in range(2)]
                nc.vector.tensor_tensor(t13[0][:], ps[1][:], osb[:], ADD)
                nc.vector.tensor_tensor(t13[1][:], ps[1][:], osb[:], SUB)
                nc.scalar.activation(
                    o1[:, m, rows_u(1)], t13[0][:], RELU, bias=b1[:, m:m + 1])
                nc.scalar.activation(
                    o1[:, m, rows_u(3)], t13[1][:], RELU, bias=b1[:, m:m + 1])

            # ---- parity prep for L2 ----
            nc.vector.tensor_tensor(
                sd[:, :, 0:TOK], o1[:, :, rows_u(1)], o1[:, :, rows_u(3)], ADD)
            nc.vector.tensor_tensor(
                sd[:, :, TOK:2 * TOK], o1[:, :, rows_u(1)], o1[:, :, rows_u(3)], SUB)

            # ---- stage L2 + softshrink + stage F, fused per block ----
            for b in range(NB):
                ps0 = ppool.tile([BS, RC], F32, tag="ps")
                ps2 = ppool.tile([BS, RC], F32, tag="ps")
                pse = ppool.tile([BS, RC], F32, tag="ps")
                pso = ppool.tile([BS, RC], F32, tag="ps")
                nc.tensor.matmul(ps0[:], W2s[:, b, :], o1[:, b, rows_u(0)],
                                 start=True, stop=True)
                nc.tensor.matmul(ps2[:], W2s[:, b, :], o1[:, b, rows_u(2)],
                                 start=True, stop=True)
                nc.tensor.matmul(pse[:], W2p[:, b, :], sd[:, b, 0:TOK],
                                 start=True, stop=True)
                nc.tensor.matmul(pso[:], W2m[:, b, :], sd[:, b, TOK:2 * TOK],
                                 start=True, stop=True)

                # v holds o2 + b2 per unit (bf16); t is the clip; z = v - t
                zt = opool.tile([BS, ROWS], BF16, tag="zt", name=f"zt_{b}")
                for u, src in ((0, ps0), (2, ps2)):
                    v = tpool.tile([BS, RC], BF16, tag="v")
                    nc.scalar.activation(v[:], src[:], IDENT, bias=b2[:, b:b + 1])
                    t = tpool.tile([BS, RC], BF16, tag="t")
                    nc.vector.tensor_scalar(t[:], v[:], -LAM, LAM, MAX, MIN)
                    nc.vector.tensor_tensor(zt[:, rows_u(u)], v[:], t[:], SUB)
                osb2 = tpool.tile([BS, RC], F32, tag="osb2", name=f"osb2_{b}")
                nc.scalar.activation(osb2[:], pso[:], IDENT)
                for u, op in ((1, ADD), (3, SUB)):
                    v = tpool.tile([BS, RC], BF16, tag="v")
                    ve = tpool.tile([BS, RC], F32, tag="ve")
                    nc.vector.tensor_tensor(ve[:], pse[:], osb2[:], op)
                    nc.scalar.activation(v[:], ve[:], IDENT, bias=b2[:, b:b + 1])
                    t = tpool.tile([BS, RC], BF16, tag="t")
                    nc.vector.tensor_scalar(t[:], v[:], -LAM, LAM, MAX, MIN)
                    nc.vector.tensor_tensor(zt[:, rows_u(u)], v[:], t[:], SUB)

                # stage F for this block: out = zt @ H128s + x
                ob = opool.tile([BS, ROWS], BF16, tag="ob", name=f"ob_{b}")
                for rc in range(NRC):
                    rs = bass.ds(rc * RC, RC)
                    psf = ppool.tile([BS, RC], F32, tag="ps")
                    nc.tensor.matmul(psf[:], H128s[:], zt[:, rs],
                                     start=True, stop=True)
                    u = rc  # RC == TOK: row chunks are units
                    if u == 0 or u == 2:
                        res = xp[:, b, rows_u(u)]
                    elif u == 1:
                        res = xr[:, b, 0:TOK]
                    else:
                        res = xr[:, b, TOK:2 * TOK]
                    nc.vector.tensor_tensor(ob[:, rs], psf[:], res, ADD)
                nc.sync.dma_start(out_ext[:, b, :], ob[:])

    nc.compile()
    return nc


_CACHED = {}


def _get_nc():
    if "nc" not in _CACHED:
        _CACHED["nc"] = build_nc()
    return _CACHED["nc"]


def _make_in_maps(x, w1, b1, w2, b2):
    wd = _prep_weights(w1, b1, w2, b2)

    # host-side shard + parity + transpose + bf16 cast
    xf = np.asarray(x, dtype=np.float32)
    units = np.empty((B, N, HID), np.float32)
    units[0] = xf[0]
    units[1] = 0.5 * (xf[1] + xf[3])
    units[2] = xf[2]
    units[3] = 0.5 * (xf[1] - xf[3])

    in_maps = []
    for c in range(NCORES):
        sl = units[:, c * TOK:(c + 1) * TOK, :]         # [4, TOK, 1024]
        # rows r = u*TOK + tok ; SBUF layout [128, NB, ROWS]
        xT = sl.reshape(ROWS, HID).T                    # [1024, 2048]
        xp = np.ascontiguousarray(
            xT.reshape(NB, BS, ROWS).transpose(1, 0, 2).astype(ml_dtypes.bfloat16))
        x13 = np.stack([xf[1, c * TOK:(c + 1) * TOK, :],
                        xf[3, c * TOK:(c + 1) * TOK, :]])   # [2, TOK, 1024]
        xrT = x13.reshape(2 * TOK, HID).T
        xr = np.ascontiguousarray(
            xrT.reshape(NB, BS, 2 * TOK).transpose(1, 0, 2).astype(ml_dtypes.bfloat16))
        m = {"xp": xp, "xr": xr}
        m.update(wd)
        in_maps.append(m)
    return in_maps


def kernel(x, w1, b1, w2, b2):
    out_dtype = x.dtype
    in_maps = _make_in_maps(x, w1, b1, w2, b2)
    nc = _get_nc()
    res = run_bass_kernel_spmd(nc, in_maps, core_ids=list(range(NCORES)))

    out = np.empty((B, N, HID), np.float32)
    for c in range(NCORES):
        ob = np.asarray(res.results[c]["out"], dtype=np.float32)  # [128, 8, 2048]
        full = ob.transpose(1, 0, 2).reshape(HID, ROWS).T         # [2048, 1024]
        out[:, c * TOK:(c + 1) * TOK, :] = full.reshape(B, TOK, HID)
    return out.astype(out_dtype)
